# revision 1
# baseline (speedup 1.0000x reference)
"""Causal self-attention with anchor-relative rope (ferope), 8-core TRN2 Bass kernel.

Full-scale problem: B=2, T=2048, C=2048, H=16, D=128, M=32.

Sharding (tensor-parallel heads + data-parallel batch):
  - 8 cores = 2 batch groups x 4 cores. Core (b, g) handles batch b, heads 4g..4g+3.
  - qkv projection: each core computes q/k/v only for its heads (w_attn column shard),
    from x[b] transposed (host prep) so the contraction dim c sits on partitions.
  - attention computed with scores transposed: s_T[ki,qi], so both attention
    matmuls contract along partitions with no on-device transposes.
  - y_T head slices ([128, T] per head, c-major) are AllGathered within each
    4-core batch group -> y_all [C, T].
  - output projection is column-sharded: each core computes out[b][:, g*512:(g+1)*512].

All matmuls run as float32r (1 cycle/row at N>=512) except the qkv projection,
whose inputs (xT and w_attn shards) are cast to bf16 on device to fit SBUF.
"""

import math

import numpy as np

import concourse.bass as bass
import concourse.mybir as mybir
import concourse.tile as tile
from concourse import bacc
from concourse.bass_utils import run_bass_kernel_spmd

F32 = mybir.dt.float32
F32R = mybir.dt.float32r
BF16 = mybir.dt.bfloat16

# full-scale dims (hardcoded per harness contract)
B, T, C, H, DH, M = 2, 2048, 2048, 16, 128, 32
N_CORES = 8
GROUPS = 2                     # batch groups
CPG = N_CORES // GROUPS        # cores per group = 4
HPC = H // CPG                 # heads per core = 4
C_LOC = HPC * DH               # 512: per-core head channels
PANEL = 512                    # qi panel width (one psum bank)
KB = 128                       # ki block (partition dim)


def r(ap):
    """View a float32 AP as float32r for full-rate matmul."""
    return ap.bitcast(F32R)


def build_program(T=T, C=C, HPC=HPC, DH=DH, M=M, n_cores=N_CORES, groups=GROUPS):
    """Build the SPMD Bass program (same program on all cores; data differs)."""
    cpg = n_cores // groups
    c_loc = HPC * DH
    n_cb = C // KB            # contraction blocks for qkv/proj
    n_panels = T // PANEL
    n_tb = T // KB
    kb_per_panel = PANEL // KB  # 4
    inv_sqrt_d = 1.0 / math.sqrt(DH)

    nc = bacc.Bacc("TRN2", target_bir_lowering=False, debug=False,
                   num_devices=n_cores)

    xT_d = nc.dram_tensor("xT", [C, T], F32, kind="ExternalInput").ap()
    wqk_d = nc.dram_tensor("wqk", [C, 2 * c_loc], F32, kind="ExternalInput").ap()
    wv_d = nc.dram_tensor("wv", [C, c_loc], F32, kind="ExternalInput").ap()
    wo_d = nc.dram_tensor("wo", [C, c_loc], F32, kind="ExternalInput").ap()
    freqs_d = nc.dram_tensor("freqs", [M], F32, kind="ExternalInput").ap()
    delta_d = nc.dram_tensor("delta", [T], F32, kind="ExternalInput").ap()
    out_d = nc.dram_tensor("out", [T, c_loc], F32, kind="ExternalOutput").ap()

    replica_groups = [list(range(g * cpg, (g + 1) * cpg)) for g in range(groups)]

    with tile.TileContext(nc) as tc:
        with (
            tc.tile_pool(name="dram", bufs=1, space="DRAM") as dram,
            tc.tile_pool(name="const", bufs=1) as const,
            tc.tile_pool(name="qkv", bufs=1) as qkv,
            tc.tile_pool(name="work", bufs=1) as work,
        ):
            y_part = dram.tile([c_loc, T], BF16)
            y_all = dram.tile([cpg * c_loc, T], BF16)

            # ---- constants: trig tables, causal masks, ones ----
            ones128 = const.tile([KB, KB], BF16)
            nc.vector.memset(ones128[:], 1.0)

            sinN = const.tile([2 * M, T], F32)
            cos64 = const.tile([2 * M, T], F32)
            masks = [const.tile([KB, PANEL], BF16, name=f"maskf{p}")
                     for p in range(kb_per_panel)]
            with tc.tile_pool(name="setup", bufs=1) as setup:
                # fr64 = [-freqs; freqs] as per-partition scalars
                fr64 = setup.tile([2 * M, 1], F32)
                nc.sync.dma_start(out=fr64[0:M, :],
                                  in_=freqs_d.rearrange("m -> m ()"))
                nc.sync.dma_start(out=fr64[M:2 * M, :],
                                  in_=freqs_d.rearrange("m -> m ()"))
                nc.vector.tensor_scalar_mul(fr64[0:M, :], fr64[0:M, :], -1.0)

                # delta broadcast across 2M partitions
                delta_row = setup.tile([1, T], F32)
                nc.sync.dma_start(out=delta_row[:],
                                  in_=delta_d.rearrange("t -> () t"))
                delta_rep = setup.tile([2 * M, T], F32)
                nc.gpsimd.partition_broadcast(delta_rep[:], delta_row[:],
                                              channels=2 * M)

                # ang = delta * (+-freqs); sinN = [-sin; sin], cos = [cos; cos]
                ang = setup.tile([2 * M, T], F32)
                nc.vector.tensor_scalar_mul(ang[:], delta_rep[:], fr64[:])
                nc.scalar.activation(sinN[:], ang[:],
                                     mybir.ActivationFunctionType.Sin)
                pi2 = setup.tile([2 * M, 1], F32)
                nc.vector.memset(pi2[:], math.pi / 2)
                nc.scalar.activation(cos64[:], ang[:],
                                     mybir.ActivationFunctionType.Sin,
                                     bias=pi2[:])

                # causal masks for diagonal tiles: mask_p = (qi >= ki + 128*p)
                for p in range(kb_per_panel):
                    mi = setup.tile([KB, PANEL], F32, tag="maski", bufs=2,
                                    name=f"maski{p}")
                    nc.gpsimd.iota(mi[:], pattern=[[1, PANEL]], base=-KB * p,
                                   channel_multiplier=-1,
                                   allow_small_or_imprecise_dtypes=True)
                    nc.vector.tensor_scalar(masks[p][:], mi[:], 0.0, None,
                                            mybir.AluOpType.is_ge)

            # ---- qkv projection for all local heads, single pass over xT ----
            # q/k stored per head as [d, t] bf16; v natural [t, d] bf16.
            q_sb = [qkv.tile([DH, T], BF16, name=f"q{h}") for h in range(HPC)]
            k_sb = [qkv.tile([DH, T], BF16, name=f"k{h}") for h in range(HPC)]
            v_all = qkv.tile([KB, n_tb, c_loc], BF16)
            with tc.tile_pool(name="wload", bufs=1) as wload:
                STAGE_ELEMS = 4 * 512  # f32 staging slot: 8KB/partition

                def load_bf16(dst3, src_t, width, name):
                    """Chunked DRAM->SBUF load of a [C, width] slab (kb-tiled
                    3D view src_t [p, kb, width]) into bf16 tile dst3,
                    chunking along kb so DMA rows stay >=2KB."""
                    kbc = min(max(STAGE_ELEMS // width, 1), n_cb)
                    for ci in range((n_cb + kbc - 1) // kbc):
                        lo = ci * kbc
                        hi = min(lo + kbc, n_cb)
                        st = wload.tile([KB, hi - lo, width], F32,
                                        tag="stage3", bufs=3,
                                        name=f"st_{name}{ci}")
                        nc.sync.dma_start(out=st[:], in_=src_t[:, lo:hi, :])
                        nc.vector.tensor_copy(dst3[:, lo:hi, :], st[:])

                wv_t = wv_d.rearrange("(kb p) c -> p kb c", p=KB)
                wvb3 = wload.tile([KB, n_cb, c_loc], BF16, tag="wvbf")
                load_bf16(wvb3, wv_t, c_loc, "wv")
                wvb = [wvb3[:, i, :] for i in range(n_cb)]
                wqk_t = wqk_d.rearrange("(kb p) c -> p kb c", p=KB)
                wqkb3 = wload.tile([KB, n_cb, 2 * c_loc], BF16, tag="wqkbf")
                load_bf16(wqkb3, wqk_t, 2 * c_loc, "wqk")
                wqkb = [wqkb3[:, i, :] for i in range(n_cb)]
                xT_t = xT_d.rearrange("(kb p) t -> p kb t", p=KB)
                with tc.tile_pool(name="psq", bufs=1, space="PSUM") as psq:
                    for tp in range(n_panels):
                        tps = tp * PANEL
                        xb3 = wload.tile([KB, n_cb, PANEL], BF16, tag="xbf",
                                         bufs=2, name=f"xb{tp}")
                        kbc = min(max(STAGE_ELEMS // PANEL, 1), n_cb)
                        for ci in range((n_cb + kbc - 1) // kbc):
                            lo = ci * kbc
                            hi = min(lo + kbc, n_cb)
                            st = wload.tile([KB, hi - lo, PANEL], F32,
                                            tag="stage3", bufs=3,
                                            name=f"st_x{tp}_{ci}")
                            nc.gpsimd.dma_start(
                                out=st[:],
                                in_=xT_t[:, lo:hi, tps:tps + PANEL])
                            nc.scalar.copy(xb3[:, lo:hi, :], st[:])
                        xbf = [xb3[:, kb, :] for kb in range(n_cb)]
                        # v blocks for the 128-rows inside this panel
                        for tbl in range(kb_per_panel):
                            tb = tp * kb_per_panel + tbl
                            pv = psq.tile([KB, c_loc], F32, tag="v", bufs=3)
                            for kb in range(n_cb):
                                nc.tensor.matmul(
                                    pv[:],
                                    xbf[kb][:, tbl * KB:(tbl + 1) * KB],
                                    wvb[kb],
                                    start=(kb == 0), stop=(kb == n_cb - 1))
                            nc.scalar.copy(v_all[:, tb, :], pv[:])
                        # q/k column blocks: cb<HPC -> q head cb; else k head
                        for cb in range(2 * HPC):
                            pqk = psq.tile([DH, PANEL], F32, tag="qk", bufs=3)
                            for kb in range(n_cb):
                                nc.tensor.matmul(
                                    pqk[:],
                                    wqkb[kb][:, cb * DH:(cb + 1) * DH],
                                    xbf[kb],
                                    start=(kb == 0), stop=(kb == n_cb - 1))
                            dst = q_sb[cb] if cb < HPC else k_sb[cb - HPC]
                            nc.scalar.copy(dst[:, tps:tps + PANEL], pqk[:])

            # ---- rope on rows 0:2M of each q/k head ----
            for u in [t for pair in zip(q_sb, k_sb) for t in pair]:
                sw = work.tile([2 * M, T], BF16, tag="ropesw", bufs=2)
                nc.vector.tensor_copy(sw[0:M, :], u[M:2 * M, :])
                nc.vector.tensor_copy(sw[M:2 * M, :], u[0:M, :])
                nc.vector.tensor_mul(sw[:], sw[:], sinN[:])
                nc.vector.tensor_mul(u[0:2 * M, :], u[0:2 * M, :], cos64[:])
                nc.vector.tensor_add(u[0:2 * M, :], u[0:2 * M, :], sw[:])

            # ---- causal attention per head + per-head AllGather ----
            # y_all rows are head-major: (head, group, d) so each per-head
            # gather writes one contiguous [cpg*DH, T] block
            with tc.tile_pool(name="proj", bufs=1) as proj:
                # prefetch proj weights during attention
                wob = []
                for i in range(n_cb):
                    wo_st = proj.tile([KB, c_loc], F32, tag="wost", bufs=2,
                                      name=f"wost{i}")
                    nc.sync.dma_start(out=wo_st[:],
                                      in_=wo_d[i * KB:(i + 1) * KB, :])
                    wo_sb = proj.tile([KB, c_loc], BF16, tag="wo", bufs=n_cb,
                                      name=f"wo{i}")
                    nc.vector.tensor_copy(wo_sb[:], wo_st[:])
                    wob.append(wo_sb)
                psa_cm = tc.tile_pool(name="psa", bufs=1, space="PSUM")
                psa = psa_cm.__enter__()
                pso_cm = tc.tile_pool(name="pso", bufs=1, space="PSUM")
                pso = pso_cm.__enter__()
                # SBUF f32 accumulators for the output projection
                out_acc = [proj.tile([KB, c_loc], F32, name=f"oacc{i}")
                           for i in range(n_tb)]
                # y_all row (hh, g, p) -> c-block cb = g*HPC + hh
                y_all_tiled = y_all[:].rearrange(
                    "(hh g p) t -> p hh g t", hh=HPC, g=cpg)

                def proj_chunk(hh):
                    """Accumulate head-chunk hh of the output projection;
                    overlaps with later heads' attention + gathers."""
                    for tb in range(n_tb):
                        yt = proj.tile([KB, cpg, KB], BF16, tag="yt", bufs=4)
                        nc.sync.dma_start(
                            out=yt[:],
                            in_=y_all_tiled[:, hh, :, tb * KB:(tb + 1) * KB])
                        po = pso.tile([KB, c_loc], F32, tag="po", bufs=2)
                        for g in range(cpg):
                            nc.tensor.matmul(po[:], yt[:, g, :],
                                             wob[g * HPC + hh][:],
                                             start=(g == 0),
                                             stop=(g == cpg - 1))
                        if hh == 0:
                            nc.vector.tensor_copy(out_acc[tb][:], po[:])
                        else:
                            nc.vector.tensor_add(out_acc[tb][:],
                                                 out_acc[tb][:], po[:])
                        if hh == HPC - 1:
                            nc.sync.dma_start(
                                out=out_d[tb * KB:(tb + 1) * KB, :],
                                in_=out_acc[tb][:])

                for h in range(HPC):
                    qh, kh = q_sb[h], k_sb[h]
                    for J in range(n_panels):
                        nkb = (J + 1) * kb_per_panel
                        py = psa.tile([DH, PANEL], F32, tag="y", bufs=3)
                        pr = psa.tile([KB, PANEL], F32, tag="r", bufs=1)
                        qs = J * PANEL
                        for b in range(nkb):
                            ps = psa.tile([KB, PANEL], F32, tag="s", bufs=2)
                            nc.tensor.matmul(
                                ps[:],
                                kh[:, b * KB:(b + 1) * KB],
                                qh[:, qs:qs + PANEL],
                                start=True, stop=True)
                            et = work.tile([KB, PANEL], BF16, tag="exp",
                                           bufs=4)
                            nc.scalar.activation(
                                et[:], ps[:],
                                mybir.ActivationFunctionType.Exp,
                                scale=inv_sqrt_d)
                            p = b - kb_per_panel * J
                            if p >= 0:
                                nc.vector.tensor_mul(et[:], et[:], masks[p][:])
                            nc.tensor.matmul(
                                py[:],
                                v_all[:, b, h * DH:(h + 1) * DH],
                                et[:],
                                start=(b == 0), stop=(b == nkb - 1))
                            # rowsum, pre-replicated across partitions by
                            # using an all-ones [128,128] stationary operand
                            nc.tensor.matmul(
                                pr[:], ones128[:], et[:],
                                start=(b == 0), stop=(b == nkb - 1))
                        # normalize: y * (1/rowsum)
                        rep = work.tile([KB, PANEL], F32, tag="rep_sb", bufs=2)
                        nc.scalar.copy(rep[:], pr[:])
                        rinv = work.tile([KB, PANEL], F32, tag="rinv", bufs=2)
                        nc.vector.reciprocal_approx_fast(rinv[:], rep[:])
                        ysb = work.tile([DH, PANEL], BF16, tag="ysb", bufs=3)
                        nc.vector.tensor_mul(ysb[:], py[:], rinv[:])
                        nc.sync.dma_start(
                            out=y_part[h * DH:(h + 1) * DH, qs:qs + PANEL],
                            in_=ysb[:])
                    # gather this head's y slice across the batch group
                    nc.gpsimd.collective_compute(
                        "AllGather",
                        mybir.AluOpType.bypass,
                        replica_groups=replica_groups,
                        ins=[y_part[h * DH:(h + 1) * DH, :]],
                        outs=[y_all[h * cpg * DH:(h + 1) * cpg * DH, :]],
                    )
                for h in range(HPC):
                    proj_chunk(h)

                pso_cm.__exit__(None, None, None)
                psa_cm.__exit__(None, None, None)

    nc.compile()
    return nc


def make_in_maps(x, w_attn, w_proj, freqs, delta, n_cores=N_CORES,
                 groups=GROUPS, dh=DH):
    """Host-side sharding: slice/transpose full inputs into per-core maps."""
    x = np.asarray(x, dtype=np.float32)
    w_attn = np.asarray(w_attn, dtype=np.float32)
    w_proj = np.asarray(w_proj, dtype=np.float32)
    freqs = np.asarray(freqs, dtype=np.float32)
    delta = np.asarray(delta, dtype=np.float32)
    b_, t_, c_ = x.shape
    cpg = n_cores // groups
    h_ = w_attn.shape[1] // (3 * dh)
    hpc = h_ // cpg
    c_loc = hpc * dh
    in_maps = []
    for core in range(n_cores):
        g, pos = divmod(core, cpg)
        heads = range(pos * hpc, (pos + 1) * hpc)
        xT = np.ascontiguousarray(x[g].T)
        wqk = np.concatenate(
            [w_attn[:, h * dh:(h + 1) * dh] for h in heads]
            + [w_attn[:, c_ + h * dh:c_ + (h + 1) * dh] for h in heads], axis=1)
        wv = np.concatenate(
            [w_attn[:, 2 * c_ + h * dh:2 * c_ + (h + 1) * dh] for h in heads],
            axis=1)
        wo = np.ascontiguousarray(w_proj[:, pos * c_loc:(pos + 1) * c_loc])
        in_maps.append({
            "xT": xT,
            "wqk": np.ascontiguousarray(wqk),
            "wv": np.ascontiguousarray(wv),
            "wo": wo,
            "freqs": freqs,
            "delta": delta,
        })
    return in_maps


def assemble_output(results, n_cores=N_CORES, groups=GROUPS):
    cpg = n_cores // groups
    outs = []
    for g in range(groups):
        cols = [results[g * cpg + pos]["out"] for pos in range(cpg)]
        outs.append(np.concatenate(cols, axis=1))
    return np.stack(outs, axis=0).astype(np.float32)


_NC_CACHE = {}


def _get_program():
    if "nc" not in _NC_CACHE:
        _NC_CACHE["nc"] = build_program()
    return _NC_CACHE["nc"]


def kernel(x, w_attn, w_proj, freqs, delta):
    nc = _get_program()
    in_maps = make_in_maps(x, w_attn, w_proj, freqs, delta)
    res = run_bass_kernel_spmd(nc, in_maps, list(range(N_CORES)))
    return assemble_output(res.results)



# revision 3
# speedup vs baseline: 1.1664x; 1.1664x over previous
"""Causal self-attention with anchor-relative rope (ferope), 8-core TRN2 Bass kernel.

Full-scale problem: B=2, T=2048, C=2048, H=16, D=128, M=32.

Sharding (tensor-parallel heads + data-parallel batch):
  - 8 cores = 2 batch groups x 4 cores. Core (g, pos) handles batch g, local
    heads 0..3 = global heads pos*4..pos*4+3.
  - All weights/x are cast to bf16 on the HOST and DMA'd directly (no on-device
    staging/casting). Rope sin/cos tables and causal masks are also
    host-precomputed (bf16) so the device setup phase is pure DMA.
  - qkv: per panel of 512 timesteps, kb-outer accumulation so matmuls start as
    soon as the first DMA chunks land. Rope is fused per panel on the DVE in
    bf16 right after each q/k psum->sbuf copy.
  - attention uses transposed scores s_T[ki,qi]; causal structure exploited at
    128-column granularity on diagonal blocks (sub-window matmuls/exp).
  - rowsum of exp via ones-stationary matmul accumulated in PSUM.
  - y slices AllGathered per head within each 4-core batch group (Shared-output
    collectives); head 3 is gathered in two T-halves so the output projection
    tail overlaps the last gather.
  - output projection is column-sharded; accumulated over head-chunks in SBUF
    f32 so each chunk only needs its own head's gather.
"""

import math

import numpy as np
import ml_dtypes

import concourse.bass as bass
import concourse.mybir as mybir
import concourse.tile as tile
from concourse import bacc
from concourse.bass_utils import run_bass_kernel_spmd

F32 = mybir.dt.float32
BF16 = mybir.dt.bfloat16

# full-scale dims (hardcoded per harness contract)
B, T, C, H, DH, M = 2, 2048, 2048, 16, 128, 32
N_CORES = 8
GROUPS = 2                     # batch groups
CPG = N_CORES // GROUPS        # cores per group = 4
HPC = H // CPG                 # heads per core = 4
C_LOC = HPC * DH               # 512: per-core head channels
PANEL = 512                    # qi panel width (one psum bank)
KB = 128                       # ki block (partition dim)
N_CB = C // KB                 # 16 contraction blocks
N_TB = T // KB                 # 16 timestep blocks
N_PANELS = T // PANEL          # 4
KB_PER_PANEL = PANEL // KB     # 4


def build_program():
    """Build the SPMD Bass program (same program on all cores; data differs)."""
    inv_sqrt_d = 1.0 / math.sqrt(DH)

    nc = bacc.Bacc("TRN2", target_bir_lowering=False, debug=False,
                   num_devices=N_CORES)

    xT_d = nc.dram_tensor("xT", [C, T], BF16, kind="ExternalInput").ap()
    wqk_d = nc.dram_tensor("wqk", [C, 2 * C_LOC], BF16, kind="ExternalInput").ap()
    wv_d = nc.dram_tensor("wv", [C, C_LOC], BF16, kind="ExternalInput").ap()
    wo_d = nc.dram_tensor("wo", [C, C_LOC], BF16, kind="ExternalInput").ap()
    tab_d = nc.dram_tensor("tab", [2 * M, 2, T], BF16, kind="ExternalInput").ap()
    masks_d = nc.dram_tensor("masks", [KB, KB_PER_PANEL, PANEL], BF16,
                             kind="ExternalInput").ap()
    out_d = nc.dram_tensor("out", [T, C_LOC], F32, kind="ExternalOutput").ap()

    replica_groups = [list(range(g * CPG, (g + 1) * CPG)) for g in range(GROUPS)]

    # partition-tiled views of the contraction dim
    xT_t = xT_d.rearrange("(kb p) t -> p kb t", p=KB)
    wqk_t = wqk_d.rearrange("(kb p) c -> p kb c", p=KB)
    wv_t = wv_d.rearrange("(kb p) c -> p kb c", p=KB)
    wo_t = wo_d.rearrange("(kb p) c -> p kb c", p=KB)

    with tile.TileContext(nc) as tc:
        with (
            tc.tile_pool(name="dram", bufs=1, space="DRAM") as dram,
            tc.tile_pool(name="const", bufs=1) as const,
            tc.tile_pool(name="qkv", bufs=1) as qkv,
            tc.tile_pool(name="work", bufs=1) as work,
        ):
            # DRAM comm buffers. Gather inputs must be Local; outputs Shared.
            y_part = dram.tile([(HPC - 1) * KB, T], BF16)       # heads 0..2
            y3a = dram.tile([KB, T // 2], BF16)                 # head 3, 1st T-half
            y3b = dram.tile([KB, T // 2], BF16)                 # head 3, 2nd T-half
            y_all = dram.tile([(HPC - 1) * CPG * KB, T], BF16)
            y3a_all = dram.tile([CPG * KB, T // 2], BF16)
            y3b_all = dram.tile([CPG * KB, T // 2], BF16)

            # ---- constants: DMA'd host-precomputed tables ----
            # tab[:, 0, :] = [-sin; +sin], tab[:, 1, :] = [cos; cos]
            tab_sb = const.tile([2 * M, 2, T], BF16)
            nc.gpsimd.dma_start(out=tab_sb[:], in_=tab_d)
            ones128 = const.tile([KB, KB], BF16)
            nc.vector.memset(ones128[:], 1.0)

            # weights (host-cast bf16), chunked DMAs for fine-grained deps
            wv_sb = const.tile([KB, N_CB, C_LOC], BF16)
            for ci in range(2):
                nc.sync.dma_start(out=wv_sb[:, 8 * ci:8 * ci + 8, :],
                                  in_=wv_t[:, 8 * ci:8 * ci + 8, :])
            wqk_sb = const.tile([KB, N_CB, 2 * C_LOC], BF16)
            for ci in range(8):
                nc.sync.dma_start(out=wqk_sb[:, 2 * ci:2 * ci + 2, :],
                                  in_=wqk_t[:, 2 * ci:2 * ci + 2, :])
            masks_sb = const.tile([KB, KB_PER_PANEL, PANEL], BF16)
            nc.sync.dma_start(out=masks_sb[:], in_=masks_d)

            # ---- qkv projection + fused rope ----
            # q/k stored per head as [d, t] bf16; v natural [t, d] bf16.
            q_sb = [qkv.tile([DH, T], BF16, name=f"q{h}") for h in range(HPC)]
            k_sb = [qkv.tile([DH, T], BF16, name=f"k{h}") for h in range(HPC)]
            v_all = qkv.tile([KB, N_TB, C_LOC], BF16)
            # cb -> destination tile: [q0, k0, q1, k1, q2, k2, q3, k3]
            qk_dst = [t for pair in zip(q_sb, k_sb) for t in pair]

            def rope(dst, tps):
                """In-place ferope on rows 0:2M of dst[:, tps:tps+PANEL]."""
                s = dst[0:2 * M, tps:tps + PANEL]
                sw = work.tile([2 * M, PANEL], BF16, tag="sw", bufs=3)
                nc.vector.tensor_copy(sw[0:M, :], dst[M:2 * M, tps:tps + PANEL])
                nc.vector.tensor_copy(sw[M:2 * M, :], dst[0:M, tps:tps + PANEL])
                nc.vector.tensor_mul(sw[:], sw[:], tab_sb[:, 0, tps:tps + PANEL])
                nc.vector.tensor_mul(s, s, tab_sb[:, 1, tps:tps + PANEL])
                nc.vector.tensor_add(s, s, sw[:])

            with tc.tile_pool(name="xpool", bufs=1) as xpool:
                xbs = []
                for tp in range(N_PANELS):
                    xb = xpool.tile([KB, N_CB, PANEL], BF16, tag="xb", bufs=2,
                                    name=f"xb{tp}")
                    tps = tp * PANEL
                    if tp == 0:
                        # fine chunks so the first matmuls start early
                        for ci in range(4):
                            nc.gpsimd.dma_start(
                                out=xb[:, 4 * ci:4 * ci + 4, :],
                                in_=xT_t[:, 4 * ci:4 * ci + 4, tps:tps + PANEL])
                    else:
                        nc.gpsimd.dma_start(out=xb[:],
                                            in_=xT_t[:, :, tps:tps + PANEL])
                    xbs.append(xb)

                with tc.tile_pool(name="psq", bufs=1, space="PSUM") as psq:
                    for tp in range(N_PANELS):
                        xb = xbs[tp]
                        tps = tp * PANEL
                        # pass A: v blocks, kb-outer
                        pvs = [psq.tile([KB, C_LOC], F32, tag="pv", bufs=4,
                                        name=f"pv{tp}_{t}") for t in range(4)]
                        for kb in range(N_CB):
                            for tbl in range(KB_PER_PANEL):
                                nc.tensor.matmul(
                                    pvs[tbl][:],
                                    xb[:, kb, tbl * KB:(tbl + 1) * KB],
                                    wv_sb[:, kb, :],
                                    start=(kb == 0), stop=(kb == N_CB - 1))
                        for tbl in range(KB_PER_PANEL):
                            nc.scalar.copy(v_all[:, tp * KB_PER_PANEL + tbl, :],
                                           pvs[tbl][:])
                        # pass B: q/k column blocks in two halves, kb-outer
                        for half in range(2):
                            pqks = [psq.tile([DH, PANEL], F32, tag="pqk",
                                             bufs=4, name=f"pqk{tp}_{half}_{j}")
                                    for j in range(4)]
                            for kb in range(N_CB):
                                for j in range(4):
                                    cb = half * 4 + j
                                    nc.tensor.matmul(
                                        pqks[j][:],
                                        wqk_sb[:, kb, cb * DH:(cb + 1) * DH],
                                        xb[:, kb, :],
                                        start=(kb == 0), stop=(kb == N_CB - 1))
                            for j in range(4):
                                cb = half * 4 + j
                                dst = qk_dst[cb]
                                nc.scalar.copy(dst[:, tps:tps + PANEL],
                                               pqks[j][:])
                                rope(dst, tps)

            # ---- causal attention per head + per-head AllGather ----
            with tc.tile_pool(name="proj", bufs=1) as proj:
                # prefetch proj weights during attention
                wo_sb = proj.tile([KB, N_CB, C_LOC], BF16)
                for ci in range(2):
                    nc.sync.dma_start(out=wo_sb[:, 8 * ci:8 * ci + 8, :],
                                      in_=wo_t[:, 8 * ci:8 * ci + 8, :])

                psa_cm = tc.tile_pool(name="psa", bufs=1, space="PSUM")
                psa = psa_cm.__enter__()
                pso_cm = tc.tile_pool(name="pso", bufs=1, space="PSUM")
                pso = pso_cm.__enter__()
                out_acc = [proj.tile([KB, C_LOC], F32, name=f"oacc{i}")
                           for i in range(N_TB)]

                for h in range(HPC):
                    qh, kh = q_sb[h], k_sb[h]
                    for J in range(N_PANELS):
                        nkb = (J + 1) * KB_PER_PANEL
                        qs = J * PANEL
                        py = psa.tile([DH, PANEL], F32, tag="y", bufs=2)
                        pr = psa.tile([KB, PANEL], F32, tag="r", bufs=2)
                        for b in range(nkb):
                            p = b - KB_PER_PANEL * J
                            o = KB * p if p > 0 else 0  # causal col window
                            ps = psa.tile([KB, PANEL], F32, tag="s", bufs=2)
                            nc.tensor.matmul(
                                ps[:, o:PANEL],
                                kh[:, b * KB:(b + 1) * KB],
                                qh[:, qs + o:qs + PANEL],
                                start=True, stop=True)
                            et = work.tile([KB, PANEL], BF16, tag="exp",
                                           bufs=4)
                            nc.scalar.activation(
                                et[:, o:PANEL], ps[:, o:PANEL],
                                mybir.ActivationFunctionType.Exp,
                                scale=inv_sqrt_d)
                            if p >= 0:
                                nc.vector.tensor_mul(
                                    et[:, o:PANEL], et[:, o:PANEL],
                                    masks_sb[:, p, o:PANEL])
                            nc.tensor.matmul(
                                py[:, o:PANEL],
                                v_all[:, b, h * DH:(h + 1) * DH],
                                et[:, o:PANEL],
                                start=(b == 0), stop=(b == nkb - 1))
                            nc.tensor.matmul(
                                pr[:, o:PANEL], ones128[:], et[:, o:PANEL],
                                start=(b == 0), stop=(b == nkb - 1))
                        # normalize: y * (1/rowsum)
                        rinv = work.tile([KB, PANEL], F32, tag="rinv", bufs=2)
                        nc.vector.reciprocal_approx_fast(rinv[:], pr[:])
                        ysb = work.tile([DH, PANEL], BF16, tag="ysb", bufs=3)
                        nc.vector.tensor_mul(ysb[:], py[:], rinv[:])
                        if h < HPC - 1:
                            nc.gpsimd.dma_start(
                                out=y_part[h * DH:(h + 1) * DH,
                                           qs:qs + PANEL],
                                in_=ysb[:])
                        else:
                            yh = y3a if J < 2 else y3b
                            cs = qs - (0 if J < 2 else T // 2)
                            nc.gpsimd.dma_start(
                                out=yh[:, cs:cs + PANEL], in_=ysb[:])
                            if J == 1:
                                nc.gpsimd.collective_compute(
                                    "AllGather", mybir.AluOpType.bypass,
                                    replica_groups=replica_groups,
                                    ins=[y3a[:]], outs=[y3a_all[:]])
                    if h < HPC - 1:
                        nc.gpsimd.collective_compute(
                            "AllGather", mybir.AluOpType.bypass,
                            replica_groups=replica_groups,
                            ins=[y_part[h * DH:(h + 1) * DH, :]],
                            outs=[y_all[h * CPG * DH:(h + 1) * CPG * DH, :]])
                    else:
                        nc.gpsimd.collective_compute(
                            "AllGather", mybir.AluOpType.bypass,
                            replica_groups=replica_groups,
                            ins=[y3b[:]], outs=[y3b_all[:]])

                # ---- output projection, head-chunk-major for gather overlap
                y_all_t = y_all[:].rearrange("(hh g p) t -> p hh g t",
                                             hh=HPC - 1, g=CPG)
                y3a_t = y3a_all[:].rearrange("(g p) t -> p g t", g=CPG)
                y3b_t = y3b_all[:].rearrange("(g p) t -> p g t", g=CPG)

                for hh in range(HPC):
                    for tb in range(N_TB):
                        yt = work.tile([KB, CPG, KB], BF16, tag="yt", bufs=4)
                        if hh < HPC - 1:
                            src = y_all_t[:, hh, :, tb * KB:(tb + 1) * KB]
                        elif tb < N_TB // 2:
                            src = y3a_t[:, :, tb * KB:(tb + 1) * KB]
                        else:
                            tbb = tb - N_TB // 2
                            src = y3b_t[:, :, tbb * KB:(tbb + 1) * KB]
                        nc.gpsimd.dma_start(out=yt[:], in_=src)
                        po = pso.tile([KB, C_LOC], F32, tag="po", bufs=2)
                        for g in range(CPG):
                            nc.tensor.matmul(po[:], yt[:, g, :],
                                             wo_sb[:, g * HPC + hh, :],
                                             start=(g == 0),
                                             stop=(g == CPG - 1))
                        if hh == 0:
                            nc.vector.tensor_copy(out_acc[tb][:], po[:])
                        else:
                            nc.vector.tensor_add(out_acc[tb][:],
                                                 out_acc[tb][:], po[:])
                        if hh == HPC - 1:
                            nc.sync.dma_start(
                                out=out_d[tb * KB:(tb + 1) * KB, :],
                                in_=out_acc[tb][:])

                pso_cm.__exit__(None, None, None)
                psa_cm.__exit__(None, None, None)

    nc.compile()
    return nc


def make_in_maps(x, w_attn, w_proj, freqs, delta):
    """Host-side sharding: slice/transpose/cast full inputs into per-core maps."""
    bf16 = ml_dtypes.bfloat16
    x = np.asarray(x, dtype=np.float32)
    w_attn = np.asarray(w_attn, dtype=np.float32)
    w_proj = np.asarray(w_proj, dtype=np.float32)
    freqs = np.asarray(freqs, dtype=np.float32)
    delta = np.asarray(delta, dtype=np.float32)

    # rope tables: tab[0:M,0] = -sin, tab[M:2M,0] = +sin, tab[:,1] = cos
    ang = delta[:, None].astype(np.float64) * freqs[None, :].astype(np.float64)
    sin_t = np.sin(ang).T.astype(np.float32)   # [M, T]
    cos_t = np.cos(ang).T.astype(np.float32)
    tab = np.empty((2 * M, 2, T), np.float32)
    tab[0:M, 0] = -sin_t
    tab[M:2 * M, 0] = sin_t
    tab[0:M, 1] = cos_t
    tab[M:2 * M, 1] = cos_t
    tab = tab.astype(bf16)

    # causal masks: masks[ki, p, qi] = 1 if qi >= ki + 128*p
    ki = np.arange(KB)[:, None, None]
    pp = np.arange(KB_PER_PANEL)[None, :, None]
    qi = np.arange(PANEL)[None, None, :]
    masks = (qi >= ki + KB * pp).astype(bf16)

    in_maps = []
    for core in range(N_CORES):
        g, pos = divmod(core, CPG)
        heads = range(pos * HPC, (pos + 1) * HPC)
        xT = np.ascontiguousarray(x[g].T.astype(bf16))
        # cb order: q0, k0, q1, k1, q2, k2, q3, k3 (local heads)
        wqk_cols = []
        for h in heads:
            wqk_cols.append(w_attn[:, h * DH:(h + 1) * DH])
            wqk_cols.append(w_attn[:, C + h * DH:C + (h + 1) * DH])
        wqk = np.ascontiguousarray(np.concatenate(wqk_cols, axis=1).astype(bf16))
        wv = np.ascontiguousarray(np.concatenate(
            [w_attn[:, 2 * C + h * DH:2 * C + (h + 1) * DH] for h in heads],
            axis=1).astype(bf16))
        wo = np.ascontiguousarray(
            w_proj[:, pos * C_LOC:(pos + 1) * C_LOC].astype(bf16))
        in_maps.append({
            "xT": xT,
            "wqk": wqk,
            "wv": wv,
            "wo": wo,
            "tab": tab,
            "masks": masks,
        })
    return in_maps


def assemble_output(results):
    outs = []
    for g in range(GROUPS):
        cols = [results[g * CPG + pos]["out"] for pos in range(CPG)]
        outs.append(np.concatenate(cols, axis=1))
    return np.stack(outs, axis=0).astype(np.float32)


_NC_CACHE = {}


def _get_program():
    if "nc" not in _NC_CACHE:
        _NC_CACHE["nc"] = build_program()
    return _NC_CACHE["nc"]


def kernel(x, w_attn, w_proj, freqs, delta):
    nc = _get_program()
    in_maps = make_in_maps(x, w_attn, w_proj, freqs, delta)
    res = run_bass_kernel_spmd(nc, in_maps, list(range(N_CORES)))
    return assemble_output(res.results)


# revision 6
# speedup vs baseline: 1.1894x; 1.0197x over previous
"""Causal self-attention with anchor-relative rope (ferope), 8-core TRN2 Bass kernel.

Full-scale problem: B=2, T=2048, C=2048, H=16, D=128, M=32.

Sharding (tensor-parallel heads + data-parallel batch):
  - 8 cores = 2 batch groups x 4 cores. Core (g, pos) handles batch g, local
    heads 0..3 = global heads pos*4..pos*4+3.
  - All weights/x are cast to bf16 on the HOST and DMA'd directly (no on-device
    staging/casting). Rope sin/cos tables and causal masks are also
    host-precomputed (bf16) so the device setup phase is pure DMA.
  - qkv: per panel of 512 timesteps, kb-outer accumulation so matmuls start as
    soon as the first DMA chunks land. Rope is fused per panel on the DVE in
    bf16 right after each q/k psum->sbuf copy.
  - attention uses transposed scores s_T[ki,qi]; causal structure exploited at
    128-column granularity on diagonal blocks (sub-window matmuls/exp).
  - rowsum of exp via ones-stationary matmul accumulated in PSUM.
  - y slices AllGathered per head within each 4-core batch group (Shared-output
    collectives); head 3 is gathered in two T-halves so the output projection
    tail overlaps the last gather.
  - output projection is column-sharded; accumulated over head-chunks in SBUF
    f32 so each chunk only needs its own head's gather.
"""

import math

import numpy as np
import ml_dtypes

import concourse.bass as bass
import concourse.mybir as mybir
import concourse.tile as tile
from concourse import bacc
from concourse.bass_utils import run_bass_kernel_spmd

F32 = mybir.dt.float32
BF16 = mybir.dt.bfloat16

# full-scale dims (hardcoded per harness contract)
B, T, C, H, DH, M = 2, 2048, 2048, 16, 128, 32
N_CORES = 8
GROUPS = 2                     # batch groups
CPG = N_CORES // GROUPS        # cores per group = 4
HPC = H // CPG                 # heads per core = 4
C_LOC = HPC * DH               # 512: per-core head channels
PANEL = 512                    # qi panel width (one psum bank)
KB = 128                       # ki block (partition dim)
N_CB = C // KB                 # 16 contraction blocks
N_TB = T // KB                 # 16 timestep blocks
N_PANELS = T // PANEL          # 4
KB_PER_PANEL = PANEL // KB     # 4


def build_program():
    """Build the SPMD Bass program (same program on all cores; data differs)."""
    inv_sqrt_d = 1.0 / math.sqrt(DH)

    nc = bacc.Bacc("TRN2", target_bir_lowering=False, debug=False,
                   num_devices=N_CORES)

    xT_d = nc.dram_tensor("xT", [C, T], BF16, kind="ExternalInput").ap()
    wqk_d = nc.dram_tensor("wqk", [C, 2 * C_LOC], BF16, kind="ExternalInput").ap()
    wv_d = nc.dram_tensor("wv", [C, C_LOC], BF16, kind="ExternalInput").ap()
    wo_d = nc.dram_tensor("wo", [C, C_LOC], BF16, kind="ExternalInput").ap()
    tab_d = nc.dram_tensor("tab", [2 * M, 2, T], BF16, kind="ExternalInput").ap()
    masks_d = nc.dram_tensor("masks", [KB, KB_PER_PANEL, PANEL], BF16,
                             kind="ExternalInput").ap()
    out_d = nc.dram_tensor("out", [T, C_LOC], F32, kind="ExternalOutput").ap()

    replica_groups = [list(range(g * CPG, (g + 1) * CPG)) for g in range(GROUPS)]

    # partition-tiled views of the contraction dim
    xT_t = xT_d.rearrange("(kb p) t -> p kb t", p=KB)
    wqk_t = wqk_d.rearrange("(kb p) c -> p kb c", p=KB)
    wv_t = wv_d.rearrange("(kb p) c -> p kb c", p=KB)
    wo_t = wo_d.rearrange("(kb p) c -> p kb c", p=KB)

    with tile.TileContext(nc) as tc:
        with (
            tc.tile_pool(name="dram", bufs=1, space="DRAM") as dram,
            tc.tile_pool(name="const", bufs=1) as const,
            tc.tile_pool(name="qkv", bufs=1) as qkv,
            tc.tile_pool(name="work", bufs=1) as work,
        ):
            # DRAM comm buffers, one per (head, T-half) so gathers start early
            # and are fine-grained enough to overlap attention + projection.
            y_parts = [[dram.tile([KB, T // 2], BF16, name=f"yp{h}_{hf}")
                        for hf in range(2)] for h in range(HPC)]
            y_alls = [[dram.tile([CPG * KB, T // 2], BF16, name=f"ya{h}_{hf}")
                       for hf in range(2)] for h in range(HPC)]

            # weights (host-cast bf16), chunked DMAs for fine-grained deps
            wv_sb = const.tile([KB, N_CB, C_LOC], BF16)
            for ci in range(2):
                nc.sync.dma_start(out=wv_sb[:, 8 * ci:8 * ci + 8, :],
                                  in_=wv_t[:, 8 * ci:8 * ci + 8, :])
            wqk_sb = const.tile([KB, N_CB, 2 * C_LOC], BF16)
            for ci in range(8):
                nc.sync.dma_start(out=wqk_sb[:, 2 * ci:2 * ci + 2, :],
                                  in_=wqk_t[:, 2 * ci:2 * ci + 2, :])
            # tab[:, 0, :] = [-sin; +sin], tab[:, 1, :] = [cos; cos]
            tab_sb = const.tile([2 * M, 2, T], BF16)
            nc.sync.dma_start(out=tab_sb[:], in_=tab_d)
            ones128 = const.tile([KB, KB], BF16)
            nc.vector.memset(ones128[:], 1.0)
            masks_sb = const.tile([KB, KB_PER_PANEL, PANEL], BF16)
            nc.sync.dma_start(out=masks_sb[:], in_=masks_d)

            # ---- qkv projection + fused rope ----
            # q/k stored per head as [d, t] bf16; v natural [t, d] bf16.
            q_sb = [qkv.tile([DH, T], BF16, name=f"q{h}") for h in range(HPC)]
            k_sb = [qkv.tile([DH, T], BF16, name=f"k{h}") for h in range(HPC)]
            v_all = qkv.tile([KB, N_TB, C_LOC], BF16)
            # cb -> destination tile: [q0, k0, q1, k1, q2, k2, q3, k3]
            qk_dst = [t for pair in zip(q_sb, k_sb) for t in pair]

            def rope(dst, tps):
                """In-place ferope on rows 0:2M of dst[:, tps:tps+PANEL]."""
                s = dst[0:2 * M, tps:tps + PANEL]
                sw = work.tile([2 * M, PANEL], BF16, tag="sw", bufs=3)
                nc.vector.tensor_copy(sw[0:M, :], dst[M:2 * M, tps:tps + PANEL])
                nc.vector.tensor_copy(sw[M:2 * M, :], dst[0:M, tps:tps + PANEL])
                nc.vector.tensor_mul(sw[:], sw[:], tab_sb[:, 0, tps:tps + PANEL])
                nc.vector.tensor_mul(s, s, tab_sb[:, 1, tps:tps + PANEL])
                nc.vector.tensor_add(s, s, sw[:])

            with tc.tile_pool(name="xpool", bufs=1) as xpool:
                xbs = []
                for tp in range(N_PANELS):
                    xb = xpool.tile([KB, N_CB, PANEL], BF16, tag="xb", bufs=2,
                                    name=f"xb{tp}")
                    tps = tp * PANEL
                    if tp == 0:
                        # fine chunks so the first matmuls start early
                        for ci in range(4):
                            nc.gpsimd.dma_start(
                                out=xb[:, 4 * ci:4 * ci + 4, :],
                                in_=xT_t[:, 4 * ci:4 * ci + 4, tps:tps + PANEL])
                    else:
                        nc.gpsimd.dma_start(out=xb[:],
                                            in_=xT_t[:, :, tps:tps + PANEL])
                    xbs.append(xb)

                with tc.tile_pool(name="psq", bufs=1, space="PSUM") as psq:
                    for tp in range(N_PANELS):
                        xb = xbs[tp]
                        tps = tp * PANEL
                        # pass A: v blocks, kb-outer
                        pvs = [psq.tile([KB, C_LOC], F32, tag="pv", bufs=4,
                                        name=f"pv{tp}_{t}") for t in range(4)]
                        for kb in range(N_CB):
                            for tbl in range(KB_PER_PANEL):
                                nc.tensor.matmul(
                                    pvs[tbl][:],
                                    xb[:, kb, tbl * KB:(tbl + 1) * KB],
                                    wv_sb[:, kb, :],
                                    start=(kb == 0), stop=(kb == N_CB - 1))
                        for tbl in range(KB_PER_PANEL):
                            nc.scalar.copy(v_all[:, tp * KB_PER_PANEL + tbl, :],
                                           pvs[tbl][:])
                        # pass B: q/k column blocks in two halves, kb-outer
                        for half in range(2):
                            pqks = [psq.tile([DH, PANEL], F32, tag="pqk",
                                             bufs=4, name=f"pqk{tp}_{half}_{j}")
                                    for j in range(4)]
                            for kb in range(N_CB):
                                for j in range(4):
                                    cb = half * 4 + j
                                    nc.tensor.matmul(
                                        pqks[j][:],
                                        wqk_sb[:, kb, cb * DH:(cb + 1) * DH],
                                        xb[:, kb, :],
                                        start=(kb == 0), stop=(kb == N_CB - 1))
                            for j in range(4):
                                cb = half * 4 + j
                                dst = qk_dst[cb]
                                nc.scalar.copy(dst[:, tps:tps + PANEL],
                                               pqks[j][:])
                                rope(dst, tps)

            # ---- causal attention per head + per-head AllGather ----
            with tc.tile_pool(name="proj", bufs=1) as proj:
                # prefetch proj weights during attention
                wo_sb = proj.tile([KB, N_CB, C_LOC], BF16)
                for ci in range(2):
                    nc.sync.dma_start(out=wo_sb[:, 8 * ci:8 * ci + 8, :],
                                      in_=wo_t[:, 8 * ci:8 * ci + 8, :])

                psa_cm = tc.tile_pool(name="psa", bufs=1, space="PSUM")
                psa = psa_cm.__enter__()
                pso_cm = tc.tile_pool(name="pso", bufs=1, space="PSUM")
                pso = pso_cm.__enter__()
                out_acc = [proj.tile([KB, C_LOC], F32, name=f"oacc{i}")
                           for i in range(N_TB)]

                for h in range(HPC):
                    qh, kh = q_sb[h], k_sb[h]
                    for J in range(N_PANELS):
                        nkb = (J + 1) * KB_PER_PANEL
                        qs = J * PANEL
                        py = psa.tile([DH, PANEL], F32, tag="y", bufs=2)
                        pr = psa.tile([KB, PANEL], F32, tag="r", bufs=2)
                        for b in range(nkb):
                            p = b - KB_PER_PANEL * J
                            o = KB * p if p > 0 else 0  # causal col window
                            ps = psa.tile([KB, PANEL], F32, tag="s", bufs=2)
                            nc.tensor.matmul(
                                ps[:, o:PANEL],
                                kh[:, b * KB:(b + 1) * KB],
                                qh[:, qs + o:qs + PANEL],
                                start=True, stop=True)
                            et = work.tile([KB, PANEL], BF16, tag="exp",
                                           bufs=4)
                            nc.scalar.activation(
                                et[:, o:PANEL], ps[:, o:PANEL],
                                mybir.ActivationFunctionType.Exp,
                                scale=inv_sqrt_d)
                            if p >= 0:
                                nc.vector.tensor_mul(
                                    et[:, o:PANEL], et[:, o:PANEL],
                                    masks_sb[:, p, o:PANEL])
                            nc.tensor.matmul(
                                py[:, o:PANEL],
                                v_all[:, b, h * DH:(h + 1) * DH],
                                et[:, o:PANEL],
                                start=(b == 0), stop=(b == nkb - 1))
                            nc.tensor.matmul(
                                pr[:, o:PANEL], ones128[:], et[:, o:PANEL],
                                start=(b == 0), stop=(b == nkb - 1))
                        # normalize: y * (1/rowsum)
                        rinv = work.tile([KB, PANEL], F32, tag="rinv", bufs=2)
                        nc.vector.reciprocal_approx_fast(rinv[:], pr[:])
                        ysb = work.tile([DH, PANEL], BF16, tag="ysb", bufs=3)
                        nc.vector.tensor_mul(ysb[:], py[:], rinv[:])
                        hf = J // 2
                        cs = qs - hf * (T // 2)
                        nc.gpsimd.dma_start(
                            out=y_parts[h][hf][:, cs:cs + PANEL], in_=ysb[:])
                        if J % 2 == 1:
                            nc.gpsimd.collective_compute(
                                "AllGather", mybir.AluOpType.bypass,
                                replica_groups=replica_groups,
                                ins=[y_parts[h][hf][:]],
                                outs=[y_alls[h][hf][:]])

                # ---- output projection, (head-chunk, T-half)-major so each
                # chunk only depends on one fine-grained gather
                y_all_ts = [[y_alls[h][hf][:].rearrange("(g p) t -> p g t",
                                                        g=CPG)
                             for hf in range(2)] for h in range(HPC)]
                nq = 0
                for hf in range(2):
                    for hh in range(HPC):
                        for ld in range(4):   # 4 loads of 2 tb each
                            yt = work.tile([KB, CPG, 2 * KB], BF16, tag="yt",
                                           bufs=6)
                            src = y_all_ts[hh][hf][:, :,
                                                   2 * ld * KB:
                                                   (2 * ld + 2) * KB]
                            q_eng = nc.gpsimd if nq % 2 == 0 else nc.sync
                            nq += 1
                            q_eng.dma_start(out=yt[:], in_=src)
                            for sub in range(2):
                                tb = hf * (N_TB // 2) + ld * 2 + sub
                                po = pso.tile([KB, C_LOC], F32, tag="po",
                                              bufs=2)
                                for g in range(CPG):
                                    nc.tensor.matmul(
                                        po[:],
                                        yt[:, g, sub * KB:(sub + 1) * KB],
                                        wo_sb[:, g * HPC + hh, :],
                                        start=(g == 0), stop=(g == CPG - 1))
                                if hh == 0:
                                    nc.vector.tensor_copy(out_acc[tb][:],
                                                          po[:])
                                else:
                                    nc.vector.tensor_add(out_acc[tb][:],
                                                         out_acc[tb][:],
                                                         po[:])
                                if hh == HPC - 1:
                                    nc.scalar.dma_start(
                                        out=out_d[tb * KB:(tb + 1) * KB, :],
                                        in_=out_acc[tb][:])

                pso_cm.__exit__(None, None, None)
                psa_cm.__exit__(None, None, None)

    nc.compile()
    return nc


def make_in_maps(x, w_attn, w_proj, freqs, delta):
    """Host-side sharding: slice/transpose/cast full inputs into per-core maps."""
    bf16 = ml_dtypes.bfloat16
    x = np.asarray(x, dtype=np.float32)
    w_attn = np.asarray(w_attn, dtype=np.float32)
    w_proj = np.asarray(w_proj, dtype=np.float32)
    freqs = np.asarray(freqs, dtype=np.float32)
    delta = np.asarray(delta, dtype=np.float32)

    # rope tables: tab[0:M,0] = -sin, tab[M:2M,0] = +sin, tab[:,1] = cos
    ang = delta[:, None].astype(np.float64) * freqs[None, :].astype(np.float64)
    sin_t = np.sin(ang).T.astype(np.float32)   # [M, T]
    cos_t = np.cos(ang).T.astype(np.float32)
    tab = np.empty((2 * M, 2, T), np.float32)
    tab[0:M, 0] = -sin_t
    tab[M:2 * M, 0] = sin_t
    tab[0:M, 1] = cos_t
    tab[M:2 * M, 1] = cos_t
    tab = tab.astype(bf16)

    # causal masks: masks[ki, p, qi] = 1 if qi >= ki + 128*p
    ki = np.arange(KB)[:, None, None]
    pp = np.arange(KB_PER_PANEL)[None, :, None]
    qi = np.arange(PANEL)[None, None, :]
    masks = (qi >= ki + KB * pp).astype(bf16)

    in_maps = []
    for core in range(N_CORES):
        g, pos = divmod(core, CPG)
        heads = range(pos * HPC, (pos + 1) * HPC)
        xT = np.ascontiguousarray(x[g].T.astype(bf16))
        # cb order: q0, k0, q1, k1, q2, k2, q3, k3 (local heads)
        wqk_cols = []
        for h in heads:
            wqk_cols.append(w_attn[:, h * DH:(h + 1) * DH])
            wqk_cols.append(w_attn[:, C + h * DH:C + (h + 1) * DH])
        wqk = np.ascontiguousarray(np.concatenate(wqk_cols, axis=1).astype(bf16))
        wv = np.ascontiguousarray(np.concatenate(
            [w_attn[:, 2 * C + h * DH:2 * C + (h + 1) * DH] for h in heads],
            axis=1).astype(bf16))
        wo = np.ascontiguousarray(
            w_proj[:, pos * C_LOC:(pos + 1) * C_LOC].astype(bf16))
        in_maps.append({
            "xT": xT,
            "wqk": wqk,
            "wv": wv,
            "wo": wo,
            "tab": tab,
            "masks": masks,
        })
    return in_maps


def assemble_output(results):
    outs = []
    for g in range(GROUPS):
        cols = [results[g * CPG + pos]["out"] for pos in range(CPG)]
        outs.append(np.concatenate(cols, axis=1))
    return np.stack(outs, axis=0).astype(np.float32)


_NC_CACHE = {}


def _get_program():
    if "nc" not in _NC_CACHE:
        _NC_CACHE["nc"] = build_program()
    return _NC_CACHE["nc"]


def kernel(x, w_attn, w_proj, freqs, delta):
    nc = _get_program()
    in_maps = make_in_maps(x, w_attn, w_proj, freqs, delta)
    res = run_bass_kernel_spmd(nc, in_maps, list(range(N_CORES)))
    return assemble_output(res.results)


# revision 10
# speedup vs baseline: 1.2894x; 1.0841x over previous
"""Causal self-attention with anchor-relative rope (ferope), 8-core TRN2 Bass kernel.

Full-scale problem: B=2, T=2048, C=2048, H=16, D=128, M=32.

Sharding (tensor-parallel heads + data-parallel batch):
  - 8 cores = 2 batch groups x 4 cores. Core (g, pos) handles batch g, local
    heads 0..3 = global heads pos*4..pos*4+3.
  - All weights/x are cast to bf16 on the HOST and DMA'd directly (no on-device
    staging/casting). Rope sin/cos tables and causal masks are also
    host-precomputed (bf16) so the device setup phase is pure DMA.
  - qkv: per panel of 512 timesteps, kb-outer accumulation so matmuls start as
    soon as the first DMA chunks land. Rope is fused per panel on the DVE in
    bf16 right after each q/k psum->sbuf copy.
  - attention uses transposed scores s_T[ki,qi]; causal structure exploited at
    128-column granularity on diagonal blocks (sub-window matmuls/exp).
  - rowsum of exp via ones-stationary matmul accumulated in PSUM.
  - y slices AllGathered per head within each 4-core batch group (Shared-output
    collectives); head 3 is gathered in two T-halves so the output projection
    tail overlaps the last gather.
  - output projection is column-sharded; accumulated over head-chunks in SBUF
    f32 so each chunk only needs its own head's gather.
"""

import math

import numpy as np
import ml_dtypes

import concourse.bass as bass
import concourse.mybir as mybir
import concourse.tile as tile
from concourse import bacc
from concourse.bass_utils import run_bass_kernel_spmd

F32 = mybir.dt.float32
BF16 = mybir.dt.bfloat16

# full-scale dims (hardcoded per harness contract)
B, T, C, H, DH, M = 2, 2048, 2048, 16, 128, 32
N_CORES = 8
GROUPS = 2                     # batch groups
CPG = N_CORES // GROUPS        # cores per group = 4
HPC = H // CPG                 # heads per core = 4
C_LOC = HPC * DH               # 512: per-core head channels
PANEL = 512                    # qi panel width (one psum bank)
KB = 128                       # ki block (partition dim)
N_CB = C // KB                 # 16 contraction blocks
N_TB = T // KB                 # 16 timestep blocks
N_PANELS = T // PANEL          # 4
KB_PER_PANEL = PANEL // KB     # 4


def build_program():
    """Build the SPMD Bass program (same program on all cores; data differs)."""
    inv_sqrt_d = 1.0 / math.sqrt(DH)

    nc = bacc.Bacc("TRN2", target_bir_lowering=False, debug=False,
                   num_devices=N_CORES)

    xT_d = nc.dram_tensor("xT", [C, T], BF16, kind="ExternalInput").ap()
    wqk_d = nc.dram_tensor("wqk", [C, 2 * C_LOC], BF16, kind="ExternalInput").ap()
    wv_d = nc.dram_tensor("wv", [C, C_LOC], BF16, kind="ExternalInput").ap()
    wo_d = nc.dram_tensor("wo", [C, C_LOC], BF16, kind="ExternalInput").ap()
    tab_d = nc.dram_tensor("tab", [2 * M, 2, T], BF16, kind="ExternalInput").ap()
    masks_d = nc.dram_tensor("masks", [KB, KB_PER_PANEL, PANEL], BF16,
                             kind="ExternalInput").ap()
    out_d = nc.dram_tensor("out", [T, C_LOC], F32, kind="ExternalOutput").ap()

    replica_groups = [list(range(g * CPG, (g + 1) * CPG)) for g in range(GROUPS)]

    # partition-tiled views of the contraction dim
    xT_t = xT_d.rearrange("(kb p) t -> p kb t", p=KB)
    wqk_t = wqk_d.rearrange("(kb p) c -> p kb c", p=KB)
    wv_t = wv_d.rearrange("(kb p) c -> p kb c", p=KB)
    wo_t = wo_d.rearrange("(kb p) c -> p kb c", p=KB)

    with tile.TileContext(nc) as tc:
        with (
            tc.tile_pool(name="dram", bufs=1, space="DRAM") as dram,
            tc.tile_pool(name="const", bufs=1) as const,
            tc.tile_pool(name="qkv", bufs=1) as qkv,
            tc.tile_pool(name="work", bufs=1) as work,
        ):
            # DRAM comm buffers, one per (head, T-half) so gathers start early
            # and are fine-grained enough to overlap attention + projection.
            y_parts = [[dram.tile([KB, T // 2], BF16, name=f"yp{h}_{hf}")
                        for hf in range(2)] for h in range(HPC)]
            y_alls = [[dram.tile([CPG * KB, T // 2], BF16, name=f"ya{h}_{hf}")
                       for hf in range(2)] for h in range(HPC)]

            # weights (host-cast bf16), chunked DMAs for fine-grained deps
            wv_sb = const.tile([KB, N_CB, C_LOC], BF16)
            for ci in range(2):
                nc.sync.dma_start(out=wv_sb[:, 8 * ci:8 * ci + 8, :],
                                  in_=wv_t[:, 8 * ci:8 * ci + 8, :])
            wqk_sb = const.tile([KB, N_CB, 2 * C_LOC], BF16)
            for ci in range(8):
                nc.sync.dma_start(out=wqk_sb[:, 2 * ci:2 * ci + 2, :],
                                  in_=wqk_t[:, 2 * ci:2 * ci + 2, :])
            # tab[:, 0, :] = [-sin; +sin], tab[:, 1, :] = [cos; cos]
            tab_sb = const.tile([2 * M, 2, T], BF16)
            nc.sync.dma_start(out=tab_sb[:], in_=tab_d)
            ones128 = const.tile([KB, KB], BF16)
            nc.vector.memset(ones128[:], 1.0)
            masks_sb = const.tile([KB, KB_PER_PANEL, PANEL], BF16)
            nc.sync.dma_start(out=masks_sb[:], in_=masks_d)

            # ---- qkv projection + fused rope ----
            # q/k stored per head as [d, t] bf16; v natural [t, d] bf16.
            q_sb = [qkv.tile([DH, T], BF16, name=f"q{h}") for h in range(HPC)]
            k_sb = [qkv.tile([DH, T], BF16, name=f"k{h}") for h in range(HPC)]
            v_all = qkv.tile([KB, N_TB, C_LOC], BF16)
            # cb -> destination tile: [q0, k0, q1, k1, q2, k2, q3, k3]
            qk_dst = [t for pair in zip(q_sb, k_sb) for t in pair]

            def rope(dst, tps):
                """In-place ferope on rows 0:2M of dst[:, tps:tps+PANEL]."""
                s = dst[0:2 * M, tps:tps + PANEL]
                sw = work.tile([2 * M, PANEL], BF16, tag="sw", bufs=3)
                nc.vector.tensor_copy(sw[0:M, :], dst[M:2 * M, tps:tps + PANEL])
                nc.vector.tensor_copy(sw[M:2 * M, :], dst[0:M, tps:tps + PANEL])
                nc.vector.tensor_mul(sw[:], sw[:], tab_sb[:, 0, tps:tps + PANEL])
                nc.vector.tensor_mul(s, s, tab_sb[:, 1, tps:tps + PANEL])
                nc.vector.tensor_add(s, s, sw[:])

            # warmup collective: absorbs the one-time ring-setup cost of the
            # first collective (~40us) while qkv matmuls run. Dummy data.
            warm_in = dram.tile([KB, 64], BF16)
            warm_out = dram.tile([CPG * KB, 64], BF16)

            with tc.tile_pool(name="xpool", bufs=1) as xpool:
                xbs = []
                for tp in range(N_PANELS):
                    xb = xpool.tile([KB, N_CB, PANEL], BF16, tag="xb", bufs=2,
                                    name=f"xb{tp}")
                    tps = tp * PANEL
                    if tp == 0:
                        # fine chunks so the first matmuls start early
                        for ci in range(4):
                            nc.gpsimd.dma_start(
                                out=xb[:, 4 * ci:4 * ci + 4, :],
                                in_=xT_t[:, 4 * ci:4 * ci + 4, tps:tps + PANEL])
                    elif tp == 1:
                        nc.gpsimd.dma_start(out=xb[:],
                                            in_=xT_t[:, :, tps:tps + PANEL])
                        # warmup sits here: gpsimd queue is blocked while the
                        # collective runs, so it must go before the xb2/xb3
                        # issues (which themselves wait on buffer reuse).
                        nc.gpsimd.collective_compute(
                            "AllGather", mybir.AluOpType.bypass,
                            replica_groups=replica_groups,
                            ins=[warm_in[:]], outs=[warm_out[:]])
                    else:
                        nc.gpsimd.dma_start(out=xb[:],
                                            in_=xT_t[:, :, tps:tps + PANEL])
                    xbs.append(xb)

                with tc.tile_pool(name="psq", bufs=1, space="PSUM") as psq:
                    for tp in range(N_PANELS):
                        xb = xbs[tp]
                        tps = tp * PANEL
                        # pass A: v blocks, kb-outer
                        pvs = [psq.tile([KB, C_LOC], F32, tag="pv", bufs=4,
                                        name=f"pv{tp}_{t}") for t in range(4)]
                        for kb in range(N_CB):
                            for tbl in range(KB_PER_PANEL):
                                nc.tensor.matmul(
                                    pvs[tbl][:],
                                    xb[:, kb, tbl * KB:(tbl + 1) * KB],
                                    wv_sb[:, kb, :],
                                    start=(kb == 0), stop=(kb == N_CB - 1))
                        for tbl in range(KB_PER_PANEL):
                            nc.scalar.copy(v_all[:, tp * KB_PER_PANEL + tbl, :],
                                           pvs[tbl][:])
                        # pass B: q/k column blocks in two halves, kb-outer
                        for half in range(2):
                            pqks = [psq.tile([DH, PANEL], F32, tag="pqk",
                                             bufs=4, name=f"pqk{tp}_{half}_{j}")
                                    for j in range(4)]
                            for kb in range(N_CB):
                                for j in range(4):
                                    cb = half * 4 + j
                                    nc.tensor.matmul(
                                        pqks[j][:],
                                        wqk_sb[:, kb, cb * DH:(cb + 1) * DH],
                                        xb[:, kb, :],
                                        start=(kb == 0), stop=(kb == N_CB - 1))
                            for j in range(4):
                                cb = half * 4 + j
                                dst = qk_dst[cb]
                                nc.scalar.copy(dst[:, tps:tps + PANEL],
                                               pqks[j][:])
                                rope(dst, tps)

            # ---- causal attention per head + per-head AllGather ----
            with tc.tile_pool(name="proj", bufs=1) as proj:
                # prefetch proj weights during attention
                wo_sb = proj.tile([KB, N_CB, C_LOC], BF16)
                for ci in range(2):
                    nc.sync.dma_start(out=wo_sb[:, 8 * ci:8 * ci + 8, :],
                                      in_=wo_t[:, 8 * ci:8 * ci + 8, :])

                psa_cm = tc.tile_pool(name="psa", bufs=1, space="PSUM")
                psa = psa_cm.__enter__()
                pso_cm = tc.tile_pool(name="pso", bufs=1, space="PSUM")
                pso = pso_cm.__enter__()
                out_acc = [proj.tile([KB, C_LOC], F32, name=f"oacc{i}")
                           for i in range(N_TB)]

                for h in range(HPC):
                    qh, kh = q_sb[h], k_sb[h]
                    for J in range(N_PANELS):
                        nkb = (J + 1) * KB_PER_PANEL
                        qs = J * PANEL
                        py = psa.tile([DH, PANEL], F32, tag="y", bufs=2)
                        pr = psa.tile([KB, PANEL], F32, tag="r", bufs=2)
                        for b in range(nkb):
                            p = b - KB_PER_PANEL * J
                            o = KB * p if p > 0 else 0  # causal col window
                            ps = psa.tile([KB, PANEL], F32, tag="s", bufs=2)
                            nc.tensor.matmul(
                                ps[:, o:PANEL],
                                kh[:, b * KB:(b + 1) * KB],
                                qh[:, qs + o:qs + PANEL],
                                start=True, stop=True)
                            et = work.tile([KB, PANEL], BF16, tag="exp",
                                           bufs=4)
                            nc.scalar.activation(
                                et[:, o:PANEL], ps[:, o:PANEL],
                                mybir.ActivationFunctionType.Exp,
                                scale=inv_sqrt_d)
                            if p >= 0:
                                nc.vector.tensor_mul(
                                    et[:, o:PANEL], et[:, o:PANEL],
                                    masks_sb[:, p, o:PANEL])
                            nc.tensor.matmul(
                                py[:, o:PANEL],
                                v_all[:, b, h * DH:(h + 1) * DH],
                                et[:, o:PANEL],
                                start=(b == 0), stop=(b == nkb - 1))
                            nc.tensor.matmul(
                                pr[:, o:PANEL], ones128[:], et[:, o:PANEL],
                                start=(b == 0), stop=(b == nkb - 1))
                        # normalize: y * (1/rowsum)
                        rinv = work.tile([KB, PANEL], F32, tag="rinv", bufs=2)
                        nc.vector.reciprocal_approx_fast(rinv[:], pr[:])
                        ysb = work.tile([DH, PANEL], BF16, tag="ysb", bufs=3)
                        nc.vector.tensor_mul(ysb[:], py[:], rinv[:])
                        hf = J // 2
                        cs = qs - hf * (T // 2)
                        nc.sync.dma_start(
                            out=y_parts[h][hf][:, cs:cs + PANEL], in_=ysb[:])
                        if J % 2 == 1:
                            nc.gpsimd.collective_compute(
                                "AllGather", mybir.AluOpType.bypass,
                                replica_groups=replica_groups,
                                ins=[y_parts[h][hf][:]],
                                outs=[y_alls[h][hf][:]])

                # ---- output projection, (head-chunk, T-half)-major so each
                # chunk only depends on one fine-grained gather
                y_all_ts = [[y_alls[h][hf][:].rearrange("(g p) t -> p g t",
                                                        g=CPG)
                             for hf in range(2)] for h in range(HPC)]
                nq = 0
                for hf in range(2):
                    for hh in range(HPC):
                        for ld in range(4):   # 4 loads of 2 tb each
                            yt = work.tile([KB, CPG, 2 * KB], BF16, tag="yt",
                                           bufs=6)
                            src = y_all_ts[hh][hf][:, :,
                                                   2 * ld * KB:
                                                   (2 * ld + 2) * KB]
                            nq += 1
                            nc.sync.dma_start(out=yt[:], in_=src)
                            for sub in range(2):
                                tb = hf * (N_TB // 2) + ld * 2 + sub
                                po = pso.tile([KB, C_LOC], F32, tag="po",
                                              bufs=2)
                                for g in range(CPG):
                                    nc.tensor.matmul(
                                        po[:],
                                        yt[:, g, sub * KB:(sub + 1) * KB],
                                        wo_sb[:, g * HPC + hh, :],
                                        start=(g == 0), stop=(g == CPG - 1))
                                if hh == 0:
                                    nc.vector.tensor_copy(out_acc[tb][:],
                                                          po[:])
                                else:
                                    nc.vector.tensor_add(out_acc[tb][:],
                                                         out_acc[tb][:],
                                                         po[:])
                                if hh == HPC - 1:
                                    nc.scalar.dma_start(
                                        out=out_d[tb * KB:(tb + 1) * KB, :],
                                        in_=out_acc[tb][:])

                pso_cm.__exit__(None, None, None)
                psa_cm.__exit__(None, None, None)

    nc.compile()
    return nc


def make_in_maps(x, w_attn, w_proj, freqs, delta):
    """Host-side sharding: slice/transpose/cast full inputs into per-core maps."""
    bf16 = ml_dtypes.bfloat16
    x = np.asarray(x, dtype=np.float32)
    w_attn = np.asarray(w_attn, dtype=np.float32)
    w_proj = np.asarray(w_proj, dtype=np.float32)
    freqs = np.asarray(freqs, dtype=np.float32)
    delta = np.asarray(delta, dtype=np.float32)

    # rope tables: tab[0:M,0] = -sin, tab[M:2M,0] = +sin, tab[:,1] = cos
    ang = delta[:, None].astype(np.float64) * freqs[None, :].astype(np.float64)
    sin_t = np.sin(ang).T.astype(np.float32)   # [M, T]
    cos_t = np.cos(ang).T.astype(np.float32)
    tab = np.empty((2 * M, 2, T), np.float32)
    tab[0:M, 0] = -sin_t
    tab[M:2 * M, 0] = sin_t
    tab[0:M, 1] = cos_t
    tab[M:2 * M, 1] = cos_t
    tab = tab.astype(bf16)

    # causal masks: masks[ki, p, qi] = 1 if qi >= ki + 128*p
    ki = np.arange(KB)[:, None, None]
    pp = np.arange(KB_PER_PANEL)[None, :, None]
    qi = np.arange(PANEL)[None, None, :]
    masks = (qi >= ki + KB * pp).astype(bf16)

    in_maps = []
    for core in range(N_CORES):
        g, pos = divmod(core, CPG)
        heads = range(pos * HPC, (pos + 1) * HPC)
        xT = np.ascontiguousarray(x[g].T.astype(bf16))
        # cb order: q0, k0, q1, k1, q2, k2, q3, k3 (local heads)
        wqk_cols = []
        for h in heads:
            wqk_cols.append(w_attn[:, h * DH:(h + 1) * DH])
            wqk_cols.append(w_attn[:, C + h * DH:C + (h + 1) * DH])
        wqk = np.ascontiguousarray(np.concatenate(wqk_cols, axis=1).astype(bf16))
        wv = np.ascontiguousarray(np.concatenate(
            [w_attn[:, 2 * C + h * DH:2 * C + (h + 1) * DH] for h in heads],
            axis=1).astype(bf16))
        wo = np.ascontiguousarray(
            w_proj[:, pos * C_LOC:(pos + 1) * C_LOC].astype(bf16))
        in_maps.append({
            "xT": xT,
            "wqk": wqk,
            "wv": wv,
            "wo": wo,
            "tab": tab,
            "masks": masks,
        })
    return in_maps


def assemble_output(results):
    outs = []
    for g in range(GROUPS):
        cols = [results[g * CPG + pos]["out"] for pos in range(CPG)]
        outs.append(np.concatenate(cols, axis=1))
    return np.stack(outs, axis=0).astype(np.float32)


_NC_CACHE = {}


def _get_program():
    if "nc" not in _NC_CACHE:
        _NC_CACHE["nc"] = build_program()
    return _NC_CACHE["nc"]


def kernel(x, w_attn, w_proj, freqs, delta):
    nc = _get_program()
    in_maps = make_in_maps(x, w_attn, w_proj, freqs, delta)
    res = run_bass_kernel_spmd(nc, in_maps, list(range(N_CORES)))
    return assemble_output(res.results)
